# revision 27
# baseline (speedup 1.0000x reference)
"""CrossDFN Trainium2 kernel.

Strategy (8 NeuronCores, SPMD, no collectives):
  - Shard: 2 images x 4 H-bands of 16 rows -> 8 shards. Host slices each
    shard WITH halo (6 rows/cols: 1 fuse + 5 max dilated-tap offset), so
    every core computes its output band independently.
  - Per-core region: "combined" pixels [18 rows x 66 cols] (16x64 output
    + 1 halo for the 3x3 fuse conv), input windows [28 x 76].
  - Tap-pair packing: partitions = (2 unfold taps, 64 ch). Host builds
    "variant" tensors per (stream, dilation r): partitions 0-63 = shard,
    64-127 = shard shifted by (0, r) columns, so one AP view reads two
    unfold taps at once (taps paired as (i,j)-(i,j+1)).
  - dwconv   = PE matmul, stacked-diagonal lhsT, PSUM-accumulate over tap
               chunks; y/x streams concurrent via col tile_position.
  - bn+relu  = ScalarE activation (per-partition scale/bias), PSUM->SBUF.
  - 1x1 conv = row-tiled K=64 matmuls (y rows 0-63, x rows 64-127
               concurrent); output channels permuted (tap-pair, ch)-major.
  - dyn mul  = VectorE scalar_tensor_tensor (K_psum + bias) * U_view, or
               (load-balanced) ScalarE evac + GPSIMD tensor_tensor.
  - tap sum  = PE matmul with stacked-identity lhsT, PSUM-accumulate into
               `combined` (x at partitions 0-63, y at 64-127).
  - fuse 3x3 = 9 shifted K=128 matmuls, PSUM-accumulate; final bn+relu.
  Matmuls run in float32r (1 cyc/row at N>=256); unfold data in bf16;
  elementwise in fp32.
"""

import os
import sys

import numpy as np

for _p in ("/opt/trn_rl_repo", "/root/.axon_site/_ro/trn_rl_repo"):
    if os.path.isdir(_p) and _p not in sys.path:
        sys.path.append(_p)

import ml_dtypes  # noqa: E402

import concourse.bass as bass  # noqa: E402
import concourse.bacc as bacc  # noqa: E402
import concourse.mybir as mybir  # noqa: E402
from concourse.tile import TileContext  # noqa: E402

BF16 = ml_dtypes.bfloat16

# ---------------------------------------------------------------- geometry
KS = [5, 5, 3, 3, 3]
RS = [1, 2, 3, 4, 5]
NB = 5
C = 64
N_IMG, H, W = 2, 64, 64
OUT_C = 64
TILE_H = 16          # output rows per core
HALO = 6             # 1 (fuse) + 5 (max tap offset)
REG_H, REG_W = 18, 66   # combined region per core
SH_H, SH_W = 28, 76     # input shard dims per core
PXC = 3              # pixel chunks along region rows
PXR = REG_H // PXC   # 6 rows -> 396 px per chunk
NPX = PXR * REG_W

ALU = mybir.AluOpType
AF = mybir.ActivationFunctionType
F32 = mybir.dt.float32
F32R = mybir.dt.float32r
DBF16 = mybir.dt.bfloat16


def branch_chunks(k):
    """Tap chunks: [(ta, tb)] pairs (tb = ta + 1 col) then (ts, None) singles."""
    pairs = [(i * k + j, i * k + j + 1) for i in range(k) for j in range(0, k - 1, 2)]
    singles = [(i * k + (k - 1), None) for i in range(k)]
    return pairs + singles


CHUNKS = [branch_chunks(k) for k in KS]
NCHUNK = sum(len(c) for c in CHUNKS)  # 48 per side


def tap_delta(t, k, r):
    c0 = (k - 1) // 2
    return ((t // k - c0) * r, (t % k - c0) * r)


# ---- dedup chunks: unique spatial offsets, paired by (0, s) col shifts;
# each chunk's dynamic filters are branch-summed in PSUM before the multiply
def _build_dedup():
    from collections import defaultdict
    offmap = defaultdict(list)
    for bi, (k, r) in enumerate(zip(KS, RS)):
        c0 = (k - 1) // 2
        for t in range(k * k):
            offmap[((t // k - c0) * r, (t % k - c0) * r)].append((bi, t))
    byrow = defaultdict(list)
    for (dr, dc) in offmap:
        byrow[dr].append(dc)
    chunks = []
    for dr in sorted(byrow):
        dcs = sorted(byrow[dr])
        i = 0
        while i < len(dcs):
            if i + 1 < len(dcs) and 1 <= dcs[i + 1] - dcs[i] <= 5:
                da, db = (dr, dcs[i]), (dr, dcs[i + 1])
                i += 2
            else:
                da, db = (dr, dcs[i]), None
                i += 1
            ca = {bi: t for bi, t in offmap[da]}
            cb = {bi: t for bi, t in offmap[db]} if db else {}
            brs = sorted(set(ca) | set(cb))
            contribs = [(bi, ca.get(bi), cb.get(bi)) for bi in brs]
            shift = db[1] - da[1] if db else 0
            chunks.append(dict(da=da, db=db, shift=shift, contribs=contribs,
                               maxbi=max(brs)))
    chunks.sort(key=lambda c: c["maxbi"])
    return chunks


DEDUP = _build_dedup()
NDD = len(DEDUP)  # 34

# pw lhsT column offsets per (dedup chunk, contributor)
DD_OFFS = []
_off = 0
for _ch in DEDUP:
    offs = []
    for _ in _ch["contribs"]:
        m = 128 if _ch["db"] is not None else 64
        offs.append((_off, m))
        _off += m
    DD_OFFS.append(offs)
PW_COLS = _off

# small-params column layout
SM_PWB_Y = 0
SM_PWB_X = SM_PWB_Y + NDD
SM_FW = SM_PWB_X + NDD
SM_BNS = SM_FW + 9 * OUT_C
SM_BNB = SM_BNS + NB
SM_NORMS = SM_BNB + NB
SM_NORMB = SM_NORMS + 1
SM_FBNS = SM_NORMB + 1
SM_FBNB = SM_FBNS + 1
SM_ID = SM_FBNB + 1
SM_COLS = SM_ID + 64


# ------------------------------------------------------------- host packing
def _pack_params(params):
    """Pack all weights into per-core-identical dense arrays."""
    p = {k: np.asarray(v, np.float32) for k, v in params.items()}

    dw_y = np.zeros((128, NCHUNK * 64), np.float32)
    dw_x = np.zeros((128, NCHUNK * 64), np.float32)
    pwW = np.zeros((128, PW_COLS), np.float32)
    small = np.zeros((128, SM_COLS), np.float32)

    qg = 0
    for bi, (k, r) in enumerate(zip(KS, RS)):
        tag = f"{k}_{r}"
        wdy = p[f"dw_y_{tag}"][:, 0].reshape(C, k * k)
        wdx = p[f"dw_x_{tag}"][:, 0].reshape(C, k * k)
        pwy = p[f"pw_y_{tag}_w"]  # [C*k*k, C]
        pwx = p[f"pw_x_{tag}_w"]
        pby = p[f"pw_y_{tag}_b"]
        pbx = p[f"pw_x_{tag}_b"]
        for qi, (ta, tb) in enumerate(CHUNKS[bi]):
            cs = qg * 64
            ar = np.arange(64)
            dw_y[ar, cs + ar] = wdy[:, ta]
            dw_x[ar, cs + ar] = wdx[:, ta]
            if tb is not None:
                dw_y[64 + ar, cs + ar] = wdy[:, tb]
                dw_x[64 + ar, cs + ar] = wdx[:, tb]
            qg += 1
        small[0:64, SM_BNS + bi] = p[f"bn_y_{tag}_s"]
        small[64:128, SM_BNS + bi] = p[f"bn_x_{tag}_s"]
        small[0:64, SM_BNB + bi] = p[f"bn_y_{tag}_b"]
        small[64:128, SM_BNB + bi] = p[f"bn_x_{tag}_b"]

    ar = np.arange(64)
    for ci, ch in enumerate(DEDUP):
        for cj, (bi, ta, tb) in enumerate(ch["contribs"]):
            k = KS[bi]
            tag = f"{k}_{RS[bi]}"
            off, m = DD_OFFS[ci][cj]
            for side, pw_w, pw_b, base in (
                ("y", p[f"pw_y_{tag}_w"], p[f"pw_y_{tag}_b"], 0),
                ("x", p[f"pw_x_{tag}_w"], p[f"pw_x_{tag}_b"], 64),
            ):
                bcol = (SM_PWB_Y if side == "y" else SM_PWB_X) + ci
                if ta is not None:
                    o = ar * (k * k) + ta
                    pwW[base : base + 64, off : off + 64] = pw_w[o, :].T
                    small[0:64, bcol] += pw_b[o]
                if tb is not None and m == 128:
                    o = ar * (k * k) + tb
                    pwW[base : base + 64, off + 64 : off + 128] = pw_w[o, :].T
                    small[64:128, bcol] += pw_b[o]

    fw = p["fuse_w"]  # [OUT_C, 2C, 3, 3]
    for ij in range(9):
        i, j = ij // 3, ij % 3
        small[:, SM_FW + ij * 64 : SM_FW + (ij + 1) * 64] = fw[:, :, i, j].T
    small[0:64, SM_NORMS] = p["norm_x_s"]
    small[64:128, SM_NORMS] = p["norm_y_s"]
    small[0:64, SM_NORMB] = p["norm_x_b"]
    small[64:128, SM_NORMB] = p["norm_y_b"]
    small[0:64, SM_FBNS] = p["fuse_bn_s"]
    small[0:64, SM_FBNB] = p["fuse_bn_b"]
    ar = np.arange(128)
    small[ar, SM_ID + ar % 64] = 1.0

    idb = np.zeros((128, 64), np.float32)
    idb[ar, ar % 64] = 1.0

    return {
        "dw_y": dw_y.astype(BF16),
        "dw_x": dw_x.astype(BF16),
        "pw_w": pwW.astype(BF16),
        "small": small,
        "id_bf": idb.astype(BF16),
    }


def _variants(img_pad, n, t):
    """img_pad: [N, C, H+12, W+17] (pad H 6/6, W 6/11). Returns per-branch
    variant arrays [128, SH_H, SH_W]: rows 0-63 base window, 64-127 shifted
    by +r columns."""
    sh = img_pad[n, :, TILE_H * t : TILE_H * t + SH_H, :]  # [C, 28, 81]
    out = []
    for r in RS:
        v = np.concatenate([sh[:, :, 0:SH_W], sh[:, :, r : r + SH_W]], axis=0)
        out.append(np.ascontiguousarray(v).astype(BF16))
    return out


def _core_inputs(ci, y, x, packed):
    n, t = ci // 4, ci % 4
    yp = np.pad(y, ((0, 0), (0, 0), (HALO, HALO), (HALO, HALO + 5)))
    xp = np.pad(x, ((0, 0), (0, 0), (HALO, HALO), (HALO, HALO + 5)))
    m = dict(packed)
    for bi, v in enumerate(_variants(yp, n, t)):
        m[f"v_y_{bi}"] = v
    for bi, v in enumerate(_variants(xp, n, t)):
        m[f"v_x_{bi}"] = v
    # fuse-conv halo rows must be zero at image boundaries (reference
    # zero-pads the 3x3 conv input); cols handled in-kernel, rows per-core.
    rm = np.ones((128, 2), np.float32)
    if t == 0:
        rm[:, 0] = 0.0
    if t == 3:
        rm[:, 1] = 0.0
    m["rowmask"] = rm
    return m


# ------------------------------------------------------------- bass kernel
def build_nc():
    nc = bacc.Bacc(trn_type="TRN2", target_bir_lowering=False)

    v_d = {}
    for s in ("y", "x"):
        for bi in range(NB):
            v_d[(s, bi)] = nc.dram_tensor(
                f"v_{s}_{bi}", [128, SH_H, SH_W], DBF16, kind="ExternalInput"
            )
    dwy_d = nc.dram_tensor("dw_y", [128, NCHUNK * 64], DBF16, kind="ExternalInput")
    dwx_d = nc.dram_tensor("dw_x", [128, NCHUNK * 64], DBF16, kind="ExternalInput")
    pww_d = nc.dram_tensor("pw_w", [128, PW_COLS], DBF16, kind="ExternalInput")
    small_d = nc.dram_tensor("small", [128, SM_COLS], F32, kind="ExternalInput")
    idb_d = nc.dram_tensor("id_bf", [128, 64], DBF16, kind="ExternalInput")
    rm_d = nc.dram_tensor("rowmask", [128, 2], F32, kind="ExternalInput")
    out_d = nc.dram_tensor("out", [64, TILE_H, W], F32, kind="ExternalOutput")

    with TileContext(nc) as tc:
        with (
            tc.tile_pool(name="const", bufs=1) as cpool,
            tc.tile_pool(name="work", bufs=1) as wpool,
            tc.tile_pool(name="psum", bufs=1, space="PSUM") as ppool,
        ):
            # ---- load constants (branch-0 data first so compute starts early)
            small = cpool.tile([128, SM_COLS], F32, name="smallt")
            nc.sync.dma_start(small[:, :], small_d[:, :])
            idb = cpool.tile([128, 64], DBF16, name="idbt")
            nc.sync.dma_start(idb[:, :], idb_d[:, :])
            rmask = cpool.tile([128, 2], F32, name="rmaskt")
            nc.sync.dma_start(rmask[:, :], rm_d[:, :])
            dwW = {}
            for s, d in (("y", dwy_d), ("x", dwx_d)):
                dt_ = cpool.tile([128, NCHUNK * 64], DBF16, name=f"dwt_{s}")
                nc.sync.dma_start(dt_[:, :], d[:, :])
                dwW[s] = dt_
            V = {}
            for bi in range(NB):
                for s in ("y", "x"):
                    vt = cpool.tile([128, SH_H, SH_W], DBF16, name=f"vt_{s}_{bi}")
                    hh_ = SH_H // 2
                    nc.sync.dma_start(vt[:, 0:hh_, :], v_d[(s, bi)][:, 0:hh_, :])
                    nc.sync.dma_start(vt[:, hh_:, :], v_d[(s, bi)][:, hh_:, :])
                    V[(s, bi)] = vt
                if bi == 0:
                    pwW = cpool.tile([128, PW_COLS], DBF16, name="pwt")
                    nc.sync.dma_start(pwW[:, :], pww_d[:, :])
            fwb = cpool.tile([128, 9 * OUT_C], DBF16, name="fwb")
            nc.vector.tensor_copy(fwb[:, :], small[:, SM_FW : SM_FW + 9 * OUT_C])

            # ---- persistent tiles
            Z = wpool.tile([128, REG_H, REG_W], DBF16, name="Z")
            comb = [
                ppool.tile([128, PXR, REG_W], F32, name=f"comb{p}", tag=f"comb{p}")
                for p in range(PXC)
            ]
            comb_started = {}  # (p, half) -> True

            # last (branch, chunk) contributes stop flags
            comb_started = {}

            def emit_dw(bi, ps):
                k, r = KS[bi], RS[bi]
                chunks = CHUNKS[bi]
                nq = len(chunks)
                if bi in Rs:
                    R = Rs[bi]
                else:
                    R = wpool.tile(
                        [128, REG_H, REG_W], DBF16, name=f"R{bi}", tag=f"R{bi}", bufs=1
                    )
                    Rs[bi] = R
                for p in ps:
                    r0 = p * PXR
                    T = ppool.tile([128, PXR, REG_W], F32, name="T", tag="T", bufs=1)
                    for qi, (ta, tb) in enumerate(chunks):
                        dr, dc = tap_delta(ta, k, r)
                        qs = (sum(len(c) for c in CHUNKS[:bi]) + qi) * 64
                        for s, cg in (("y", 0), ("x", 64)):
                            u = V[(s, bi)][
                                : (128 if tb is not None else 64),
                                r0 + 5 + dr : r0 + 5 + dr + PXR,
                                5 + dc : 5 + dc + REG_W,
                            ]
                            lhs = dwW[s][: (128 if tb is not None else 64), qs : qs + 64]
                            nc.tensor.matmul(
                                T[cg : cg + 64, :, :],
                                lhs,
                                u,
                                start=(qi == 0),
                                stop=(qi == nq - 1),
                                tile_position=(0, cg),
                                skip_group_check=True,
                            )
                    nc.scalar.activation(
                        R[:, r0 : r0 + PXR, :],
                        T[:, :, :],
                        AF.Relu,
                        bias=small[:, SM_BNB + bi : SM_BNB + bi + 1],
                        scale=small[:, SM_BNS + bi : SM_BNS + bi + 1],
                    )

            # ---------- per chunk: pw matmuls, grouped dyn-mul, tap-sum
            # dw for branch bi+1 is emitted mid-way through branch bi's chunks
            # so the PE always has independent ready work to fill stt stalls
            Rs = {}
            emitted_dw = set()
            pending_px2 = []
            for ci, chd in enumerate(DEDUP):
                # dw for a branch lands just before first use (px 0,1) with its
                # px2 group deferred into the chunk body — spreads independent
                # PE work through the loop to fill stt stalls
                for bneed in sorted(set(bi for bi, _, _ in chd["contribs"])):
                    if bneed not in emitted_dw:
                        emitted_dw.add(bneed)
                        emit_dw(bneed, [0, 1])
                        pending_px2.append(bneed)
                npart = 128 if chd["db"] is not None else 64
                dr, dc = chd["da"]
                vshift = chd["shift"] - 1  # variant index for paired taps
                contribs = chd["contribs"]
                last = ci == NDD - 1

                sides = []
                for src in ("y", "x"):
                    rt = (0, 0) if src == "y" else (64, 0)
                    other = "x" if src == "y" else "y"
                    ch = 0 if src == "y" else 64
                    bcol = (SM_PWB_Y if src == "y" else SM_PWB_X) + ci
                    sides.append(dict(
                        src=src, rt=rt, other=other, ch=ch,
                        bias=small[:npart, bcol : bcol + 1],
                    ))

                if True:

                    def pw_mm_one(sd, ktile, slot, p, cj):
                        rr0 = sd["rt"][0]
                        cbi = contribs[cj][0]
                        off, m = DD_OFFS[ci][cj]
                        nc.tensor.matmul(
                            ktile[:npart, slot, 0:NPX].rearrange(
                                "p (a b) -> p a b", a=PXR
                            ),
                            pwW[rr0 : rr0 + 64, off : off + m],
                            Rs[cbi][rr0 : rr0 + 64, p * PXR : (p + 1) * PXR, :],
                            start=(cj == 0),
                            stop=(cj == len(contribs) - 1),
                            tile_position=sd["rt"],
                            skip_group_check=True,
                        )

                    def pw_mm(sd, ktile, slot, p):
                        for cj in range(len(contribs)):
                            pw_mm_one(sd, ktile, slot, p, cj)

                    def reduce_mm(sd, pap, p):
                        fk = (sd["ch"], p)
                        fi = fk not in comb_started
                        comb_started[fk] = True
                        nc.tensor.matmul(
                            comb[p][sd["ch"] : sd["ch"] + 64, :, :],
                            idb[:npart, :],
                            pap,
                            start=fi,
                            stop=last,
                            tile_position=(0, sd["ch"]),
                            skip_group_check=True,
                        )

                    # pixel chunks 0+1 grouped; y/x interleaved so the
                    # K=64 row-tiled pw matmuls pair up on the PE array
                    K2 = {}
                    for sd in sides:
                        K2[sd["src"]] = ppool.tile(
                            [128, 2, 512], F32, name=f"K{sd['src']}",
                            tag=f"K{sd['src']}", bufs=1,
                        )
                    for slot in (0, 1):
                        for cj in range(len(contribs)):
                            for sd in sides:
                                pw_mm_one(sd, K2[sd["src"]], slot, slot, cj)
                    P2 = {}
                    for sd in sides:
                        P2[sd["src"]] = wpool.tile(
                            [128, 2, PXR, REG_W], DBF16,
                            name=f"P2{sd['src']}", tag=f"P2{sd['src']}", bufs=3,
                        )
                        u2 = V[(sd["other"], max(vshift, 0))][
                            :npart,
                            5 + dr : 5 + dr + 2 * PXR,
                            5 + dc : 5 + dc + REG_W,
                        ].rearrange("p (a b) c -> p a b c", a=2)
                        nc.vector.scalar_tensor_tensor(
                            P2[sd["src"]][:npart, :, :, :],
                            K2[sd["src"]][:npart, :, 0:NPX].rearrange(
                                "p a (b c) -> p a b c", b=PXR
                            ),
                            sd["bias"],
                            u2,
                            ALU.add,
                            ALU.mult,
                        )
                    for p in (0, 1):
                        for sd in sides:
                            reduce_mm(sd, P2[sd["src"]][:npart, p, :, :], p)
                    while pending_px2:
                        emit_dw(pending_px2.pop(0), [2])

                    # pixel chunk 2
                    K1 = {}
                    for sd in sides:
                        K1[sd["src"]] = ppool.tile(
                            [128, 2, 512], F32, name=f"K1{sd['src']}",
                            tag=f"K{sd['src']}", bufs=1,
                        )
                    for cj in range(len(contribs)):
                        for sd in sides:
                            pw_mm_one(sd, K1[sd["src"]], 0, 2, cj)
                    P1 = {}
                    for sd in sides:
                        P1[sd["src"]] = wpool.tile(
                            [128, PXR, REG_W], DBF16,
                            name=f"P1{sd['src']}", tag=f"P1{sd['src']}", bufs=3,
                        )
                        u1 = V[(sd["other"], max(vshift, 0))][
                            :npart,
                            2 * PXR + 5 + dr : 2 * PXR + 5 + dr + PXR,
                            5 + dc : 5 + dc + REG_W,
                        ]
                        nc.vector.scalar_tensor_tensor(
                            P1[sd["src"]][:npart, :, :],
                            K1[sd["src"]][:npart, 0, 0:NPX].rearrange(
                                "p (b c) -> p b c", b=PXR
                            ),
                            sd["bias"],
                            u1,
                            ALU.add,
                            ALU.mult,
                        )
                    for sd in sides:
                        reduce_mm(sd, P1[sd["src"]][:npart, :, :], 2)

            # ---------- norm + relu -> Z
            for p in range(PXC):
                nc.scalar.activation(
                    Z[:, p * PXR : (p + 1) * PXR, :],
                    comb[p][:, :, :],
                    AF.Relu,
                    bias=small[:, SM_NORMB : SM_NORMB + 1],
                    scale=small[:, SM_NORMS : SM_NORMS + 1],
                )

            # ---------- zero the fuse-conv halo ring of Z
            nc.vector.tensor_scalar_mul(Z[:, :, 0:1], Z[:, :, 0:1], 0.0)
            nc.vector.tensor_scalar_mul(
                Z[:, :, REG_W - 1 : REG_W], Z[:, :, REG_W - 1 : REG_W], 0.0
            )
            nc.vector.tensor_scalar_mul(Z[:, 0:1, :], Z[:, 0:1, :], rmask[:, 0:1])
            nc.vector.tensor_scalar_mul(
                Z[:, REG_H - 1 : REG_H, :], Z[:, REG_H - 1 : REG_H, :], rmask[:, 1:2]
            )

            # ---------- fuse 3x3 conv (K=128), two row-halves concurrent
            osb = wpool.tile([128, TILE_H, W], F32, name="osb")
            for hh in range(2):
                fps = ppool.tile(
                    [128, 8, W], F32, name=f"fps{hh}",
                    tag=f"K{'y' if hh == 0 else 'x'}", bufs=1,
                )
                cg = 0 if hh == 0 else 64
                for ij in range(9):
                    i, j = ij // 3, ij % 3
                    rr = hh * 8 + i  # Z row of out-row (hh*8) + tap i
                    nc.tensor.matmul(
                        fps[cg : cg + 64, :, :],
                        fwb[:, ij * 64 : (ij + 1) * 64],
                        Z[:, rr : rr + 8, j : j + W],
                        start=(ij == 0),
                        stop=(ij == 8),
                        tile_position=(0, cg),
                        skip_group_check=True,
                    )
                nc.scalar.activation(
                    osb[0:64, hh * 8 : (hh + 1) * 8, :],
                    fps[cg : cg + 64, :, :],
                    AF.Relu,
                    bias=small[0:64, SM_FBNB : SM_FBNB + 1],
                    scale=small[0:64, SM_FBNS : SM_FBNS + 1],
                )
            nc.sync.dma_start(out_d[:, :, :], osb[0:64, :, :])

    nc.compile()
    return nc


# ------------------------------------------------------------------ driver
LAST_RESULTS = None


def kernel(y, x, params):
    global LAST_RESULTS
    y = np.asarray(y, np.float32)
    x = np.asarray(x, np.float32)
    packed = _pack_params(params)

    in_maps = [_core_inputs(ci, y, x, packed) for ci in range(8)]
    nc = build_nc()

    from concourse.bass_utils import run_bass_kernel_spmd

    res = run_bass_kernel_spmd(
        nc,
        in_maps,
        core_ids=list(range(8)),
        trace=bool(os.environ.get("KTRACE")),
    )
    LAST_RESULTS = res

    out = np.zeros((N_IMG, OUT_C, H, W), np.float32)
    for ci in range(8):
        n, t = ci // 4, ci % 4
        out[n, :, TILE_H * t : TILE_H * (t + 1), :] = (
            np.asarray(res.results[ci]["out"], np.float32).reshape(OUT_C, TILE_H, W)
        )
    return out


# revision 28
# speedup vs baseline: 1.0682x; 1.0682x over previous
"""CrossDFN Trainium2 kernel.

Strategy (8 NeuronCores, SPMD, no collectives):
  - Shard: 2 images x 4 H-bands of 16 rows -> 8 shards. Host slices each
    shard WITH halo (6 rows/cols: 1 fuse + 5 max dilated-tap offset), so
    every core computes its output band independently.
  - Per-core region: "combined" pixels [18 rows x 66 cols] (16x64 output
    + 1 halo for the 3x3 fuse conv), input windows [28 x 76].
  - Tap-pair packing: partitions = (2 unfold taps, 64 ch). Host builds
    "variant" tensors per (stream, dilation r): partitions 0-63 = shard,
    64-127 = shard shifted by (0, r) columns, so one AP view reads two
    unfold taps at once (taps paired as (i,j)-(i,j+1)).
  - dwconv   = PE matmul, stacked-diagonal lhsT, PSUM-accumulate over tap
               chunks; y/x streams concurrent via col tile_position.
  - bn+relu  = ScalarE activation (per-partition scale/bias), PSUM->SBUF.
  - 1x1 conv = row-tiled K=64 matmuls (y rows 0-63, x rows 64-127
               concurrent); output channels permuted (tap-pair, ch)-major.
  - dyn mul  = VectorE scalar_tensor_tensor (K_psum + bias) * U_view, or
               (load-balanced) ScalarE evac + GPSIMD tensor_tensor.
  - tap sum  = PE matmul with stacked-identity lhsT, PSUM-accumulate into
               `combined` (x at partitions 0-63, y at 64-127).
  - fuse 3x3 = 9 shifted K=128 matmuls, PSUM-accumulate; final bn+relu.
  Matmuls run in float32r (1 cyc/row at N>=256); unfold data in bf16;
  elementwise in fp32.
"""

import os
import sys

import numpy as np

for _p in ("/opt/trn_rl_repo", "/root/.axon_site/_ro/trn_rl_repo"):
    if os.path.isdir(_p) and _p not in sys.path:
        sys.path.append(_p)

import ml_dtypes  # noqa: E402

import concourse.bass as bass  # noqa: E402
import concourse.bacc as bacc  # noqa: E402
import concourse.mybir as mybir  # noqa: E402
from concourse.tile import TileContext  # noqa: E402

BF16 = ml_dtypes.bfloat16

# ---------------------------------------------------------------- geometry
KS = [5, 5, 3, 3, 3]
RS = [1, 2, 3, 4, 5]
NB = 5
C = 64
N_IMG, H, W = 2, 64, 64
OUT_C = 64
TILE_H = 16          # output rows per core
HALO = 6             # 1 (fuse) + 5 (max tap offset)
REG_H, REG_W = 18, 66   # combined region per core
SH_H, SH_W = 28, 76     # input shard dims per core
PXC = 3              # pixel chunks along region rows
PXR = REG_H // PXC   # 6 rows -> 396 px per chunk
NPX = PXR * REG_W

ALU = mybir.AluOpType
AF = mybir.ActivationFunctionType
F32 = mybir.dt.float32
F32R = mybir.dt.float32r
DBF16 = mybir.dt.bfloat16


def branch_chunks(k):
    """Tap chunks: [(ta, tb)] pairs (tb = ta + 1 col) then (ts, None) singles."""
    pairs = [(i * k + j, i * k + j + 1) for i in range(k) for j in range(0, k - 1, 2)]
    singles = [(i * k + (k - 1), None) for i in range(k)]
    return pairs + singles


CHUNKS = [branch_chunks(k) for k in KS]
NCHUNK = sum(len(c) for c in CHUNKS)  # 48 per side


def tap_delta(t, k, r):
    c0 = (k - 1) // 2
    return ((t // k - c0) * r, (t % k - c0) * r)


# ---- dedup chunks: unique spatial offsets, paired by (0, s) col shifts;
# each chunk's dynamic filters are branch-summed in PSUM before the multiply
def _build_dedup():
    from collections import defaultdict
    offmap = defaultdict(list)
    for bi, (k, r) in enumerate(zip(KS, RS)):
        c0 = (k - 1) // 2
        for t in range(k * k):
            offmap[((t // k - c0) * r, (t % k - c0) * r)].append((bi, t))
    byrow = defaultdict(list)
    for (dr, dc) in offmap:
        byrow[dr].append(dc)
    chunks = []
    for dr in sorted(byrow):
        dcs = sorted(byrow[dr])
        i = 0
        while i < len(dcs):
            if i + 1 < len(dcs) and 1 <= dcs[i + 1] - dcs[i] <= 5:
                da, db = (dr, dcs[i]), (dr, dcs[i + 1])
                i += 2
            else:
                da, db = (dr, dcs[i]), None
                i += 1
            ca = {bi: t for bi, t in offmap[da]}
            cb = {bi: t for bi, t in offmap[db]} if db else {}
            brs = sorted(set(ca) | set(cb))
            contribs = [(bi, ca.get(bi), cb.get(bi)) for bi in brs]
            shift = db[1] - da[1] if db else 0
            chunks.append(dict(da=da, db=db, shift=shift, contribs=contribs,
                               maxbi=max(brs)))
    chunks.sort(key=lambda c: c["maxbi"])
    return chunks


DEDUP = _build_dedup()
NDD = len(DEDUP)  # 34

# pw lhsT column offsets per (dedup chunk, contributor)
DD_OFFS = []
_off = 0
for _ch in DEDUP:
    offs = []
    for _ in _ch["contribs"]:
        m = 128 if _ch["db"] is not None else 64
        offs.append((_off, m))
        _off += m
    DD_OFFS.append(offs)
PW_COLS = _off

# small-params column layout
SM_PWB_Y = 0
SM_PWB_X = SM_PWB_Y + NDD
SM_FW = SM_PWB_X + NDD
SM_BNS = SM_FW + 9 * OUT_C
SM_BNB = SM_BNS + NB
SM_NORMS = SM_BNB + NB
SM_NORMB = SM_NORMS + 1
SM_FBNS = SM_NORMB + 1
SM_FBNB = SM_FBNS + 1
SM_ID = SM_FBNB + 1
SM_COLS = SM_ID + 64


# ------------------------------------------------------------- host packing
def _pack_params(params):
    """Pack all weights into per-core-identical dense arrays."""
    p = {k: np.asarray(v, np.float32) for k, v in params.items()}

    dw_y = np.zeros((128, NCHUNK * 64), np.float32)
    dw_x = np.zeros((128, NCHUNK * 64), np.float32)
    pwW = np.zeros((128, PW_COLS), np.float32)
    small = np.zeros((128, SM_COLS), np.float32)

    qg = 0
    for bi, (k, r) in enumerate(zip(KS, RS)):
        tag = f"{k}_{r}"
        wdy = p[f"dw_y_{tag}"][:, 0].reshape(C, k * k)
        wdx = p[f"dw_x_{tag}"][:, 0].reshape(C, k * k)
        pwy = p[f"pw_y_{tag}_w"]  # [C*k*k, C]
        pwx = p[f"pw_x_{tag}_w"]
        pby = p[f"pw_y_{tag}_b"]
        pbx = p[f"pw_x_{tag}_b"]
        for qi, (ta, tb) in enumerate(CHUNKS[bi]):
            cs = qg * 64
            ar = np.arange(64)
            dw_y[ar, cs + ar] = wdy[:, ta]
            dw_x[ar, cs + ar] = wdx[:, ta]
            if tb is not None:
                dw_y[64 + ar, cs + ar] = wdy[:, tb]
                dw_x[64 + ar, cs + ar] = wdx[:, tb]
            qg += 1
        small[0:64, SM_BNS + bi] = p[f"bn_y_{tag}_s"]
        small[64:128, SM_BNS + bi] = p[f"bn_x_{tag}_s"]
        small[0:64, SM_BNB + bi] = p[f"bn_y_{tag}_b"]
        small[64:128, SM_BNB + bi] = p[f"bn_x_{tag}_b"]

    ar = np.arange(64)
    for ci, ch in enumerate(DEDUP):
        for cj, (bi, ta, tb) in enumerate(ch["contribs"]):
            k = KS[bi]
            tag = f"{k}_{RS[bi]}"
            off, m = DD_OFFS[ci][cj]
            for side, pw_w, pw_b, base in (
                ("y", p[f"pw_y_{tag}_w"], p[f"pw_y_{tag}_b"], 0),
                ("x", p[f"pw_x_{tag}_w"], p[f"pw_x_{tag}_b"], 64),
            ):
                bcol = (SM_PWB_Y if side == "y" else SM_PWB_X) + ci
                if ta is not None:
                    o = ar * (k * k) + ta
                    pwW[base : base + 64, off : off + 64] = pw_w[o, :].T
                    small[0:64, bcol] += pw_b[o]
                if tb is not None and m == 128:
                    o = ar * (k * k) + tb
                    pwW[base : base + 64, off + 64 : off + 128] = pw_w[o, :].T
                    small[64:128, bcol] += pw_b[o]

    fw = p["fuse_w"]  # [OUT_C, 2C, 3, 3]
    for ij in range(9):
        i, j = ij // 3, ij % 3
        small[:, SM_FW + ij * 64 : SM_FW + (ij + 1) * 64] = fw[:, :, i, j].T
    small[0:64, SM_NORMS] = p["norm_x_s"]
    small[64:128, SM_NORMS] = p["norm_y_s"]
    small[0:64, SM_NORMB] = p["norm_x_b"]
    small[64:128, SM_NORMB] = p["norm_y_b"]
    small[0:64, SM_FBNS] = p["fuse_bn_s"]
    small[0:64, SM_FBNB] = p["fuse_bn_b"]
    ar = np.arange(128)
    small[ar, SM_ID + ar % 64] = 1.0

    idb = np.zeros((128, 64), np.float32)
    idb[ar, ar % 64] = 1.0

    return {
        "dw_y": dw_y.astype(BF16),
        "dw_x": dw_x.astype(BF16),
        "pw_w": pwW.astype(BF16),
        "small": small,
        "id_bf": idb.astype(BF16),
    }


def _variants(img_pad, n, t):
    """img_pad: [N, C, H+12, W+17] (pad H 6/6, W 6/11). Returns per-branch
    variant arrays [128, SH_H, SH_W]: rows 0-63 base window, 64-127 shifted
    by +r columns."""
    sh = img_pad[n, :, TILE_H * t : TILE_H * t + SH_H, :]  # [C, 28, 81]
    out = []
    for r in RS:
        v = np.concatenate([sh[:, :, 0:SH_W], sh[:, :, r : r + SH_W]], axis=0)
        out.append(np.ascontiguousarray(v).astype(BF16))
    return out


def _core_inputs(ci, y, x, packed):
    n, t = ci // 4, ci % 4
    yp = np.pad(y, ((0, 0), (0, 0), (HALO, HALO), (HALO, HALO + 5)))
    xp = np.pad(x, ((0, 0), (0, 0), (HALO, HALO), (HALO, HALO + 5)))
    m = dict(packed)
    for bi, v in enumerate(_variants(yp, n, t)):
        m[f"v_y_{bi}"] = v
    for bi, v in enumerate(_variants(xp, n, t)):
        m[f"v_x_{bi}"] = v
    # fuse-conv halo rows must be zero at image boundaries (reference
    # zero-pads the 3x3 conv input); cols handled in-kernel, rows per-core.
    rm = np.ones((128, 2), np.float32)
    if t == 0:
        rm[:, 0] = 0.0
    if t == 3:
        rm[:, 1] = 0.0
    m["rowmask"] = rm
    return m


# ------------------------------------------------------------- bass kernel
def build_nc():
    nc = bacc.Bacc(trn_type="TRN2", target_bir_lowering=False)

    v_d = {}
    for s in ("y", "x"):
        for bi in range(NB):
            v_d[(s, bi)] = nc.dram_tensor(
                f"v_{s}_{bi}", [128, SH_H, SH_W], DBF16, kind="ExternalInput"
            )
    dwy_d = nc.dram_tensor("dw_y", [128, NCHUNK * 64], DBF16, kind="ExternalInput")
    dwx_d = nc.dram_tensor("dw_x", [128, NCHUNK * 64], DBF16, kind="ExternalInput")
    pww_d = nc.dram_tensor("pw_w", [128, PW_COLS], DBF16, kind="ExternalInput")
    small_d = nc.dram_tensor("small", [128, SM_COLS], F32, kind="ExternalInput")
    idb_d = nc.dram_tensor("id_bf", [128, 64], DBF16, kind="ExternalInput")
    rm_d = nc.dram_tensor("rowmask", [128, 2], F32, kind="ExternalInput")
    out_d = nc.dram_tensor("out", [64, TILE_H, W], F32, kind="ExternalOutput")

    with TileContext(nc) as tc:
        with (
            tc.tile_pool(name="const", bufs=1) as cpool,
            tc.tile_pool(name="work", bufs=1) as wpool,
            tc.tile_pool(name="psum", bufs=1, space="PSUM") as ppool,
        ):
            # ---- load constants (branch-0 data first so compute starts early)
            small = cpool.tile([128, SM_COLS], F32, name="smallt")
            nc.sync.dma_start(small[:, :], small_d[:, :])
            idb = cpool.tile([128, 64], DBF16, name="idbt")
            nc.sync.dma_start(idb[:, :], idb_d[:, :])
            rmask = cpool.tile([128, 2], F32, name="rmaskt")
            nc.sync.dma_start(rmask[:, :], rm_d[:, :])
            dwW = {}
            for s, d in (("y", dwy_d), ("x", dwx_d)):
                dt_ = cpool.tile([128, NCHUNK * 64], DBF16, name=f"dwt_{s}")
                nc.sync.dma_start(dt_[:, :], d[:, :])
                dwW[s] = dt_
            V = {}
            for bi in range(NB):
                for s in ("y", "x"):
                    vt = cpool.tile([128, SH_H, SH_W], DBF16, name=f"vt_{s}_{bi}")
                    nc.sync.dma_start(vt[:, :, :], v_d[(s, bi)][:, :, :])
                    V[(s, bi)] = vt
                if bi == 0:
                    pwW = cpool.tile([128, PW_COLS], DBF16, name="pwt")
                    nc.sync.dma_start(pwW[:, :], pww_d[:, :])
            fwb = cpool.tile([128, 9 * OUT_C], DBF16, name="fwb")
            nc.vector.tensor_copy(fwb[:, :], small[:, SM_FW : SM_FW + 9 * OUT_C])

            # ---- persistent tiles
            Z = wpool.tile([128, REG_H, REG_W], DBF16, name="Z")
            comb = [
                ppool.tile([128, PXR, REG_W], F32, name=f"comb{p}", tag=f"comb{p}")
                for p in range(PXC)
            ]
            comb_started = {}  # (p, half) -> True

            # last (branch, chunk) contributes stop flags
            comb_started = {}

            def emit_dw(bi, ps):
                k, r = KS[bi], RS[bi]
                chunks = CHUNKS[bi]
                nq = len(chunks)
                if bi in Rs:
                    R = Rs[bi]
                else:
                    R = wpool.tile(
                        [128, REG_H, REG_W], DBF16, name=f"R{bi}", tag=f"R{bi}", bufs=1
                    )
                    Rs[bi] = R
                for p in ps:
                    r0 = p * PXR
                    T = ppool.tile([128, PXR, REG_W], F32, name="T", tag="T", bufs=1)
                    for qi, (ta, tb) in enumerate(chunks):
                        dr, dc = tap_delta(ta, k, r)
                        qs = (sum(len(c) for c in CHUNKS[:bi]) + qi) * 64
                        for s, cg in (("y", 0), ("x", 64)):
                            u = V[(s, bi)][
                                : (128 if tb is not None else 64),
                                r0 + 5 + dr : r0 + 5 + dr + PXR,
                                5 + dc : 5 + dc + REG_W,
                            ]
                            lhs = dwW[s][: (128 if tb is not None else 64), qs : qs + 64]
                            nc.tensor.matmul(
                                T[cg : cg + 64, :, :],
                                lhs,
                                u,
                                start=(qi == 0),
                                stop=(qi == nq - 1),
                                tile_position=(0, cg),
                                skip_group_check=True,
                            )
                    nc.scalar.activation(
                        R[:, r0 : r0 + PXR, :],
                        T[:, :, :],
                        AF.Relu,
                        bias=small[:, SM_BNB + bi : SM_BNB + bi + 1],
                        scale=small[:, SM_BNS + bi : SM_BNS + bi + 1],
                    )

            # ---------- per chunk: pw matmuls, grouped dyn-mul, tap-sum
            # dw for branch bi+1 is emitted mid-way through branch bi's chunks
            # so the PE always has independent ready work to fill stt stalls
            Rs = {}
            emitted_dw = set()
            for ci, chd in enumerate(DEDUP):
                # dw for a branch is emitted just before its first use, so
                # the PE gets fresh independent work spread through the loop
                for bneed in sorted(set(bi for bi, _, _ in chd["contribs"])):
                    if bneed not in emitted_dw:
                        emitted_dw.add(bneed)
                        emit_dw(bneed, range(PXC))
                npart = 128 if chd["db"] is not None else 64
                dr, dc = chd["da"]
                vshift = chd["shift"] - 1  # variant index for paired taps
                contribs = chd["contribs"]
                last = ci == NDD - 1

                sides = []
                for src in ("y", "x"):
                    rt = (0, 0) if src == "y" else (64, 0)
                    other = "x" if src == "y" else "y"
                    ch = 0 if src == "y" else 64
                    bcol = (SM_PWB_Y if src == "y" else SM_PWB_X) + ci
                    sides.append(dict(
                        src=src, rt=rt, other=other, ch=ch,
                        bias=small[:npart, bcol : bcol + 1],
                    ))

                if True:

                    def pw_mm_one(sd, ktile, slot, p, cj):
                        rr0 = sd["rt"][0]
                        cbi = contribs[cj][0]
                        off, m = DD_OFFS[ci][cj]
                        nc.tensor.matmul(
                            ktile[:npart, slot, 0:NPX].rearrange(
                                "p (a b) -> p a b", a=PXR
                            ),
                            pwW[rr0 : rr0 + 64, off : off + m],
                            Rs[cbi][rr0 : rr0 + 64, p * PXR : (p + 1) * PXR, :],
                            start=(cj == 0),
                            stop=(cj == len(contribs) - 1),
                            tile_position=sd["rt"],
                            skip_group_check=True,
                        )

                    def pw_mm(sd, ktile, slot, p):
                        for cj in range(len(contribs)):
                            pw_mm_one(sd, ktile, slot, p, cj)

                    def reduce_mm(sd, pap, p):
                        fk = (sd["ch"], p)
                        fi = fk not in comb_started
                        comb_started[fk] = True
                        nc.tensor.matmul(
                            comb[p][sd["ch"] : sd["ch"] + 64, :, :],
                            idb[:npart, :],
                            pap,
                            start=fi,
                            stop=last,
                            tile_position=(0, sd["ch"]),
                            skip_group_check=True,
                        )

                    # pixel chunks 0+1 grouped; y/x interleaved so the
                    # K=64 row-tiled pw matmuls pair up on the PE array
                    K2 = {}
                    for sd in sides:
                        K2[sd["src"]] = ppool.tile(
                            [128, 2, 512], F32, name=f"K{sd['src']}",
                            tag=f"K{sd['src']}", bufs=1,
                        )
                    for slot in (0, 1):
                        for cj in range(len(contribs)):
                            for sd in sides:
                                pw_mm_one(sd, K2[sd["src"]], slot, slot, cj)
                    P2 = {}
                    for sd in sides:
                        P2[sd["src"]] = wpool.tile(
                            [128, 2, PXR, REG_W], DBF16,
                            name=f"P2{sd['src']}", tag=f"P2{sd['src']}", bufs=3,
                        )
                        u2 = V[(sd["other"], max(vshift, 0))][
                            :npart,
                            5 + dr : 5 + dr + 2 * PXR,
                            5 + dc : 5 + dc + REG_W,
                        ].rearrange("p (a b) c -> p a b c", a=2)
                        nc.vector.scalar_tensor_tensor(
                            P2[sd["src"]][:npart, :, :, :],
                            K2[sd["src"]][:npart, :, 0:NPX].rearrange(
                                "p a (b c) -> p a b c", b=PXR
                            ),
                            sd["bias"],
                            u2,
                            ALU.add,
                            ALU.mult,
                        )
                    for p in (0, 1):
                        for sd in sides:
                            reduce_mm(sd, P2[sd["src"]][:npart, p, :, :], p)

                    # pixel chunk 2
                    K1 = {}
                    for sd in sides:
                        K1[sd["src"]] = ppool.tile(
                            [128, 2, 512], F32, name=f"K1{sd['src']}",
                            tag=f"K{sd['src']}", bufs=1,
                        )
                    for cj in range(len(contribs)):
                        for sd in sides:
                            pw_mm_one(sd, K1[sd["src"]], 0, 2, cj)
                    P1 = {}
                    for sd in sides:
                        P1[sd["src"]] = wpool.tile(
                            [128, PXR, REG_W], DBF16,
                            name=f"P1{sd['src']}", tag=f"P1{sd['src']}", bufs=3,
                        )
                        u1 = V[(sd["other"], max(vshift, 0))][
                            :npart,
                            2 * PXR + 5 + dr : 2 * PXR + 5 + dr + PXR,
                            5 + dc : 5 + dc + REG_W,
                        ]
                        nc.vector.scalar_tensor_tensor(
                            P1[sd["src"]][:npart, :, :],
                            K1[sd["src"]][:npart, 0, 0:NPX].rearrange(
                                "p (b c) -> p b c", b=PXR
                            ),
                            sd["bias"],
                            u1,
                            ALU.add,
                            ALU.mult,
                        )
                    for sd in sides:
                        reduce_mm(sd, P1[sd["src"]][:npart, :, :], 2)

            # ---------- norm + relu -> Z
            for p in range(PXC):
                nc.scalar.activation(
                    Z[:, p * PXR : (p + 1) * PXR, :],
                    comb[p][:, :, :],
                    AF.Relu,
                    bias=small[:, SM_NORMB : SM_NORMB + 1],
                    scale=small[:, SM_NORMS : SM_NORMS + 1],
                )

            # ---------- zero the fuse-conv halo ring of Z
            nc.vector.tensor_scalar_mul(Z[:, :, 0:1], Z[:, :, 0:1], 0.0)
            nc.vector.tensor_scalar_mul(
                Z[:, :, REG_W - 1 : REG_W], Z[:, :, REG_W - 1 : REG_W], 0.0
            )
            nc.vector.tensor_scalar_mul(Z[:, 0:1, :], Z[:, 0:1, :], rmask[:, 0:1])
            nc.vector.tensor_scalar_mul(
                Z[:, REG_H - 1 : REG_H, :], Z[:, REG_H - 1 : REG_H, :], rmask[:, 1:2]
            )

            # ---------- fuse 3x3 conv (K=128), two row-halves concurrent
            osb = wpool.tile([128, TILE_H, W], F32, name="osb")
            for hh in range(2):
                fps = ppool.tile(
                    [128, 8, W], F32, name=f"fps{hh}",
                    tag=f"K{'y' if hh == 0 else 'x'}", bufs=1,
                )
                cg = 0 if hh == 0 else 64
                for ij in range(9):
                    i, j = ij // 3, ij % 3
                    rr = hh * 8 + i  # Z row of out-row (hh*8) + tap i
                    nc.tensor.matmul(
                        fps[cg : cg + 64, :, :],
                        fwb[:, ij * 64 : (ij + 1) * 64],
                        Z[:, rr : rr + 8, j : j + W],
                        start=(ij == 0),
                        stop=(ij == 8),
                        tile_position=(0, cg),
                        skip_group_check=True,
                    )
                nc.scalar.activation(
                    osb[0:64, hh * 8 : (hh + 1) * 8, :],
                    fps[cg : cg + 64, :, :],
                    AF.Relu,
                    bias=small[0:64, SM_FBNB : SM_FBNB + 1],
                    scale=small[0:64, SM_FBNS : SM_FBNS + 1],
                )
            nc.sync.dma_start(out_d[:, :, :], osb[0:64, :, :])

    nc.compile()
    return nc


# ------------------------------------------------------------------ driver
LAST_RESULTS = None


def kernel(y, x, params):
    global LAST_RESULTS
    y = np.asarray(y, np.float32)
    x = np.asarray(x, np.float32)
    packed = _pack_params(params)

    in_maps = [_core_inputs(ci, y, x, packed) for ci in range(8)]
    nc = build_nc()

    from concourse.bass_utils import run_bass_kernel_spmd

    res = run_bass_kernel_spmd(
        nc,
        in_maps,
        core_ids=list(range(8)),
        trace=bool(os.environ.get("KTRACE")),
    )
    LAST_RESULTS = res

    out = np.zeros((N_IMG, OUT_C, H, W), np.float32)
    for ci in range(8):
        n, t = ci // 4, ci % 4
        out[n, :, TILE_H * t : TILE_H * (t + 1), :] = (
            np.asarray(res.results[ci]["out"], np.float32).reshape(OUT_C, TILE_H, W)
        )
    return out


# revision 29
# speedup vs baseline: 1.1582x; 1.0842x over previous
"""CrossDFN Trainium2 kernel.

Strategy (8 NeuronCores, SPMD, no collectives):
  - Shard: 2 images x 4 H-bands of 16 rows -> 8 shards. Host slices each
    shard WITH halo (6 rows/cols: 1 fuse + 5 max dilated-tap offset), so
    every core computes its output band independently.
  - Per-core region: "combined" pixels [18 rows x 66 cols] (16x64 output
    + 1 halo for the 3x3 fuse conv), input windows [28 x 76].
  - Tap-pair packing: partitions = (2 unfold taps, 64 ch). Host builds
    "variant" tensors per (stream, dilation r): partitions 0-63 = shard,
    64-127 = shard shifted by (0, r) columns, so one AP view reads two
    unfold taps at once (taps paired as (i,j)-(i,j+1)).
  - dwconv   = PE matmul, stacked-diagonal lhsT, PSUM-accumulate over tap
               chunks; y/x streams concurrent via col tile_position.
  - bn+relu  = ScalarE activation (per-partition scale/bias), PSUM->SBUF.
  - 1x1 conv = row-tiled K=64 matmuls (y rows 0-63, x rows 64-127
               concurrent); output channels permuted (tap-pair, ch)-major.
  - dyn mul  = VectorE scalar_tensor_tensor (K_psum + bias) * U_view, or
               (load-balanced) ScalarE evac + GPSIMD tensor_tensor.
  - tap sum  = PE matmul with stacked-identity lhsT, PSUM-accumulate into
               `combined` (x at partitions 0-63, y at 64-127).
  - fuse 3x3 = 9 shifted K=128 matmuls, PSUM-accumulate; final bn+relu.
  Matmuls run in float32r (1 cyc/row at N>=256); unfold data in bf16;
  elementwise in fp32.
"""

import os
import sys

import numpy as np

for _p in ("/opt/trn_rl_repo", "/root/.axon_site/_ro/trn_rl_repo"):
    if os.path.isdir(_p) and _p not in sys.path:
        sys.path.append(_p)

import ml_dtypes  # noqa: E402

import concourse.bass as bass  # noqa: E402
import concourse.bacc as bacc  # noqa: E402
import concourse.mybir as mybir  # noqa: E402
from concourse.tile import TileContext  # noqa: E402

BF16 = ml_dtypes.bfloat16

# ---------------------------------------------------------------- geometry
KS = [5, 5, 3, 3, 3]
RS = [1, 2, 3, 4, 5]
NB = 5
C = 64
N_IMG, H, W = 2, 64, 64
OUT_C = 64
TILE_H = 16          # output rows per core
HALO = 6             # 1 (fuse) + 5 (max tap offset)
REG_H, REG_W = 18, 66   # combined region per core
SH_H, SH_W = 28, 76     # input shard dims per core
PXC = 3              # pixel chunks along region rows
PXR = REG_H // PXC   # 6 rows -> 396 px per chunk
NPX = PXR * REG_W

ALU = mybir.AluOpType
AF = mybir.ActivationFunctionType
F32 = mybir.dt.float32
F32R = mybir.dt.float32r
DBF16 = mybir.dt.bfloat16


def branch_chunks(k):
    """Tap chunks: [(ta, tb)] pairs (tb = ta + 1 col) then (ts, None) singles."""
    pairs = [(i * k + j, i * k + j + 1) for i in range(k) for j in range(0, k - 1, 2)]
    singles = [(i * k + (k - 1), None) for i in range(k)]
    return pairs + singles


CHUNKS = [branch_chunks(k) for k in KS]
NCHUNK = sum(len(c) for c in CHUNKS)  # 48 per side


def tap_delta(t, k, r):
    c0 = (k - 1) // 2
    return ((t // k - c0) * r, (t % k - c0) * r)


# ---- dedup chunks: unique spatial offsets, paired by (0, s) col shifts;
# each chunk's dynamic filters are branch-summed in PSUM before the multiply
def _build_dedup():
    from collections import defaultdict
    offmap = defaultdict(list)
    for bi, (k, r) in enumerate(zip(KS, RS)):
        c0 = (k - 1) // 2
        for t in range(k * k):
            offmap[((t // k - c0) * r, (t % k - c0) * r)].append((bi, t))
    byrow = defaultdict(list)
    for (dr, dc) in offmap:
        byrow[dr].append(dc)
    chunks = []
    for dr in sorted(byrow):
        dcs = sorted(byrow[dr])
        i = 0
        while i < len(dcs):
            if i + 1 < len(dcs) and 1 <= dcs[i + 1] - dcs[i] <= 5:
                da, db = (dr, dcs[i]), (dr, dcs[i + 1])
                i += 2
            else:
                da, db = (dr, dcs[i]), None
                i += 1
            ca = {bi: t for bi, t in offmap[da]}
            cb = {bi: t for bi, t in offmap[db]} if db else {}
            brs = sorted(set(ca) | set(cb))
            contribs = [(bi, ca.get(bi), cb.get(bi)) for bi in brs]
            shift = db[1] - da[1] if db else 0
            chunks.append(dict(da=da, db=db, shift=shift, contribs=contribs,
                               maxbi=max(brs)))
    chunks.sort(key=lambda c: c["maxbi"])
    return chunks


DEDUP = _build_dedup()
NDD = len(DEDUP)  # 34

# pw lhsT column offsets per (dedup chunk, contributor)
DD_OFFS = []
_off = 0
for _ch in DEDUP:
    offs = []
    for _ in _ch["contribs"]:
        m = 128 if _ch["db"] is not None else 64
        offs.append((_off, m))
        _off += m
    DD_OFFS.append(offs)
PW_COLS = _off

# small-params column layout
SM_PWB_Y = 0
SM_PWB_X = SM_PWB_Y + NDD
SM_FW = SM_PWB_X + NDD
SM_BNS = SM_FW + 9 * OUT_C
SM_BNB = SM_BNS + NB
SM_NORMS = SM_BNB + NB
SM_NORMB = SM_NORMS + 1
SM_FBNS = SM_NORMB + 1
SM_FBNB = SM_FBNS + 1
SM_ID = SM_FBNB + 1
SM_COLS = SM_ID + 64


# ------------------------------------------------------------- host packing
def _pack_params(params):
    """Pack all weights into per-core-identical dense arrays."""
    p = {k: np.asarray(v, np.float32) for k, v in params.items()}

    dw_y = np.zeros((128, NCHUNK * 64), np.float32)
    dw_x = np.zeros((128, NCHUNK * 64), np.float32)
    pwW = np.zeros((128, PW_COLS), np.float32)
    small = np.zeros((128, SM_COLS), np.float32)

    qg = 0
    for bi, (k, r) in enumerate(zip(KS, RS)):
        tag = f"{k}_{r}"
        wdy = p[f"dw_y_{tag}"][:, 0].reshape(C, k * k)
        wdx = p[f"dw_x_{tag}"][:, 0].reshape(C, k * k)
        pwy = p[f"pw_y_{tag}_w"]  # [C*k*k, C]
        pwx = p[f"pw_x_{tag}_w"]
        pby = p[f"pw_y_{tag}_b"]
        pbx = p[f"pw_x_{tag}_b"]
        for qi, (ta, tb) in enumerate(CHUNKS[bi]):
            cs = qg * 64
            ar = np.arange(64)
            dw_y[ar, cs + ar] = wdy[:, ta]
            dw_x[ar, cs + ar] = wdx[:, ta]
            if tb is not None:
                dw_y[64 + ar, cs + ar] = wdy[:, tb]
                dw_x[64 + ar, cs + ar] = wdx[:, tb]
            qg += 1
        small[0:64, SM_BNS + bi] = p[f"bn_y_{tag}_s"]
        small[64:128, SM_BNS + bi] = p[f"bn_x_{tag}_s"]
        small[0:64, SM_BNB + bi] = p[f"bn_y_{tag}_b"]
        small[64:128, SM_BNB + bi] = p[f"bn_x_{tag}_b"]

    ar = np.arange(64)
    for ci, ch in enumerate(DEDUP):
        for cj, (bi, ta, tb) in enumerate(ch["contribs"]):
            k = KS[bi]
            tag = f"{k}_{RS[bi]}"
            off, m = DD_OFFS[ci][cj]
            for side, pw_w, pw_b, base in (
                ("y", p[f"pw_y_{tag}_w"], p[f"pw_y_{tag}_b"], 0),
                ("x", p[f"pw_x_{tag}_w"], p[f"pw_x_{tag}_b"], 64),
            ):
                bcol = (SM_PWB_Y if side == "y" else SM_PWB_X) + ci
                if ta is not None:
                    o = ar * (k * k) + ta
                    pwW[base : base + 64, off : off + 64] = pw_w[o, :].T
                    small[0:64, bcol] += pw_b[o]
                if tb is not None and m == 128:
                    o = ar * (k * k) + tb
                    pwW[base : base + 64, off + 64 : off + 128] = pw_w[o, :].T
                    small[64:128, bcol] += pw_b[o]

    fw = p["fuse_w"]  # [OUT_C, 2C, 3, 3]
    for ij in range(9):
        i, j = ij // 3, ij % 3
        small[:, SM_FW + ij * 64 : SM_FW + (ij + 1) * 64] = fw[:, :, i, j].T
    small[0:64, SM_NORMS] = p["norm_x_s"]
    small[64:128, SM_NORMS] = p["norm_y_s"]
    small[0:64, SM_NORMB] = p["norm_x_b"]
    small[64:128, SM_NORMB] = p["norm_y_b"]
    small[0:64, SM_FBNS] = p["fuse_bn_s"]
    small[0:64, SM_FBNB] = p["fuse_bn_b"]
    ar = np.arange(128)
    small[ar, SM_ID + ar % 64] = 1.0

    idb = np.zeros((128, 64), np.float32)
    idb[ar, ar % 64] = 1.0

    return {
        "dw_y": dw_y.astype(BF16),
        "dw_x": dw_x.astype(BF16),
        "pw_w": pwW.astype(BF16),
        "small": small,
        "id_bf": idb.astype(BF16),
    }


def _variants(img_pad, n, t):
    """img_pad: [N, C, H+12, W+17] (pad H 6/6, W 6/11). Returns per-branch
    variant arrays [128, SH_H, SH_W]: rows 0-63 base window, 64-127 shifted
    by +r columns."""
    sh = img_pad[n, :, TILE_H * t : TILE_H * t + SH_H, :]  # [C, 28, 81]
    out = []
    for r in RS:
        v = np.concatenate([sh[:, :, 0:SH_W], sh[:, :, r : r + SH_W]], axis=0)
        out.append(np.ascontiguousarray(v).astype(BF16))
    return out


def _core_inputs(ci, y, x, packed):
    n, t = ci // 4, ci % 4
    yp = np.pad(y, ((0, 0), (0, 0), (HALO, HALO), (HALO, HALO + 5)))
    xp = np.pad(x, ((0, 0), (0, 0), (HALO, HALO), (HALO, HALO + 5)))
    m = dict(packed)
    for bi, v in enumerate(_variants(yp, n, t)):
        m[f"v_y_{bi}"] = v
    for bi, v in enumerate(_variants(xp, n, t)):
        m[f"v_x_{bi}"] = v
    # fuse-conv halo rows must be zero at image boundaries (reference
    # zero-pads the 3x3 conv input); cols handled in-kernel, rows per-core.
    rm = np.ones((128, 2), np.float32)
    if t == 0:
        rm[:, 0] = 0.0
    if t == 3:
        rm[:, 1] = 0.0
    m["rowmask"] = rm
    return m


# ------------------------------------------------------------- bass kernel
def build_nc():
    nc = bacc.Bacc(trn_type="TRN2", target_bir_lowering=False)

    v_d = {}
    for s in ("y", "x"):
        for bi in range(NB):
            v_d[(s, bi)] = nc.dram_tensor(
                f"v_{s}_{bi}", [128, SH_H, SH_W], DBF16, kind="ExternalInput"
            )
    dwy_d = nc.dram_tensor("dw_y", [128, NCHUNK * 64], DBF16, kind="ExternalInput")
    dwx_d = nc.dram_tensor("dw_x", [128, NCHUNK * 64], DBF16, kind="ExternalInput")
    pww_d = nc.dram_tensor("pw_w", [128, PW_COLS], DBF16, kind="ExternalInput")
    small_d = nc.dram_tensor("small", [128, SM_COLS], F32, kind="ExternalInput")
    idb_d = nc.dram_tensor("id_bf", [128, 64], DBF16, kind="ExternalInput")
    rm_d = nc.dram_tensor("rowmask", [128, 2], F32, kind="ExternalInput")
    out_d = nc.dram_tensor("out", [64, TILE_H, W], F32, kind="ExternalOutput")

    with TileContext(nc) as tc:
        with (
            tc.tile_pool(name="const", bufs=1) as cpool,
            tc.tile_pool(name="work", bufs=1) as wpool,
            tc.tile_pool(name="psum", bufs=1, space="PSUM") as ppool,
        ):
            # ---- load constants (branch-0 data first so compute starts early)
            small = cpool.tile([128, SM_COLS], F32, name="smallt")
            nc.sync.dma_start(small[:, :], small_d[:, :])
            idb = cpool.tile([128, 64], DBF16, name="idbt")
            nc.sync.dma_start(idb[:, :], idb_d[:, :])
            rmask = cpool.tile([128, 2], F32, name="rmaskt")
            nc.sync.dma_start(rmask[:, :], rm_d[:, :])
            dwW = {}
            for s, d in (("y", dwy_d), ("x", dwx_d)):
                dt_ = cpool.tile([128, NCHUNK * 64], DBF16, name=f"dwt_{s}")
                nc.sync.dma_start(dt_[:, :], d[:, :])
                dwW[s] = dt_
            V = {}
            for bi in range(NB):
                for s in ("y", "x"):
                    vt = cpool.tile([128, SH_H, SH_W], DBF16, name=f"vt_{s}_{bi}")
                    nc.sync.dma_start(vt[:, :, :], v_d[(s, bi)][:, :, :])
                    V[(s, bi)] = vt
                if bi == 0:
                    pwW = cpool.tile([128, PW_COLS], DBF16, name="pwt")
                    nc.sync.dma_start(pwW[:, :], pww_d[:, :])
            fwb = cpool.tile([128, 9 * OUT_C], DBF16, name="fwb")
            nc.vector.tensor_copy(fwb[:, :], small[:, SM_FW : SM_FW + 9 * OUT_C])

            # ---- persistent tiles
            Z = wpool.tile([128, REG_H, REG_W], DBF16, name="Z")
            comb = [
                ppool.tile([128, PXR, REG_W], F32, name=f"comb{p}", tag=f"comb{p}")
                for p in range(PXC)
            ]
            comb_started = {}  # (p, half) -> True

            # last (branch, chunk) contributes stop flags
            comb_started = {}

            def emit_dw(bi, ps):
                k, r = KS[bi], RS[bi]
                chunks = CHUNKS[bi]
                nq = len(chunks)
                if bi in Rs:
                    R = Rs[bi]
                else:
                    R = wpool.tile(
                        [128, REG_H, REG_W], DBF16, name=f"R{bi}", tag=f"R{bi}", bufs=1
                    )
                    Rs[bi] = R
                for p in ps:
                    r0 = p * PXR
                    T = ppool.tile([128, PXR, REG_W], F32, name="T", tag="T", bufs=1)
                    for qi, (ta, tb) in enumerate(chunks):
                        dr, dc = tap_delta(ta, k, r)
                        qs = (sum(len(c) for c in CHUNKS[:bi]) + qi) * 64
                        for s, cg in (("y", 0), ("x", 64)):
                            u = V[(s, bi)][
                                : (128 if tb is not None else 64),
                                r0 + 5 + dr : r0 + 5 + dr + PXR,
                                5 + dc : 5 + dc + REG_W,
                            ]
                            lhs = dwW[s][: (128 if tb is not None else 64), qs : qs + 64]
                            nc.tensor.matmul(
                                T[cg : cg + 64, :, :],
                                lhs,
                                u,
                                start=(qi == 0),
                                stop=(qi == nq - 1),
                                tile_position=(0, cg),
                                skip_group_check=True,
                            )
                    nc.scalar.activation(
                        R[:, r0 : r0 + PXR, :],
                        T[:, :, :],
                        AF.Relu,
                        bias=small[:, SM_BNB + bi : SM_BNB + bi + 1],
                        scale=small[:, SM_BNS + bi : SM_BNS + bi + 1],
                    )

            # ---------- per chunk: pw matmuls, grouped dyn-mul, tap-sum
            # dw for branch bi+1 is emitted mid-way through branch bi's chunks
            # so the PE always has independent ready work to fill stt stalls
            Rs = {}
            emitted_dw = set()
            for ci, chd in enumerate(DEDUP):
                # dw for a branch is emitted just before its first use, so
                # the PE gets fresh independent work spread through the loop
                for bneed in sorted(set(bi for bi, _, _ in chd["contribs"])):
                    if bneed not in emitted_dw:
                        emitted_dw.add(bneed)
                        emit_dw(bneed, range(PXC))
                npart = 128 if chd["db"] is not None else 64
                dr, dc = chd["da"]
                vshift = chd["shift"] - 1  # variant index for paired taps
                contribs = chd["contribs"]
                last = ci == NDD - 1

                sides = []
                for src in ("y", "x"):
                    rt = (0, 0) if src == "y" else (64, 0)
                    other = "x" if src == "y" else "y"
                    ch = 0 if src == "y" else 64
                    bcol = (SM_PWB_Y if src == "y" else SM_PWB_X) + ci
                    sides.append(dict(
                        src=src, rt=rt, other=other, ch=ch,
                        bias=small[:npart, bcol : bcol + 1],
                    ))

                if True:

                    def pw_mm_one(sd, ktile, slot, p, cj):
                        rr0 = sd["rt"][0]
                        cbi = contribs[cj][0]
                        off, m = DD_OFFS[ci][cj]
                        nc.tensor.matmul(
                            ktile[:npart, slot, 0:NPX].rearrange(
                                "p (a b) -> p a b", a=PXR
                            ),
                            pwW[rr0 : rr0 + 64, off : off + m],
                            Rs[cbi][rr0 : rr0 + 64, p * PXR : (p + 1) * PXR, :],
                            start=(cj == 0),
                            stop=(cj == len(contribs) - 1),
                            tile_position=sd["rt"],
                            skip_group_check=True,
                        )

                    def pw_mm(sd, ktile, slot, p):
                        for cj in range(len(contribs)):
                            pw_mm_one(sd, ktile, slot, p, cj)

                    def reduce_mm(sd, pap, p):
                        fk = (sd["ch"], p)
                        fi = fk not in comb_started
                        comb_started[fk] = True
                        nc.tensor.matmul(
                            comb[p][sd["ch"] : sd["ch"] + 64, :, :],
                            idb[:npart, :],
                            pap,
                            start=fi,
                            stop=last,
                            tile_position=(0, sd["ch"]),
                            skip_group_check=True,
                        )

                    # pixel chunks 0+1 grouped; y/x interleaved so the
                    # K=64 row-tiled pw matmuls pair up on the PE array
                    K2 = {}
                    for sd in sides:
                        K2[sd["src"]] = ppool.tile(
                            [128, 2, 512], F32, name=f"K{sd['src']}",
                            tag=f"K{sd['src']}", bufs=1,
                        )
                    for slot in (0, 1):
                        for cj in range(len(contribs)):
                            for sd in sides:
                                pw_mm_one(sd, K2[sd["src"]], slot, slot, cj)
                    P2 = {}
                    for sd in sides:
                        P2[sd["src"]] = wpool.tile(
                            [128, 2, PXR, REG_W], DBF16,
                            name=f"P2{sd['src']}", tag=f"P2{sd['src']}", bufs=3,
                        )
                        u2 = V[(sd["other"], max(vshift, 0))][
                            :npart,
                            5 + dr : 5 + dr + 2 * PXR,
                            5 + dc : 5 + dc + REG_W,
                        ].rearrange("p (a b) c -> p a b c", a=2)
                        nc.vector.scalar_tensor_tensor(
                            P2[sd["src"]][:npart, :, :, :],
                            K2[sd["src"]][:npart, :, 0:NPX].rearrange(
                                "p a (b c) -> p a b c", b=PXR
                            ),
                            sd["bias"],
                            u2,
                            ALU.add,
                            ALU.mult,
                        )
                    for p in (0, 1):
                        for sd in sides:
                            reduce_mm(sd, P2[sd["src"]][:npart, p, :, :], p)

                    # pixel chunk 2. Once all dw groups are emitted the
                    # T-tag bank is idle; tail chunks park K1 there instead of
                    # recycling the K2 slot, decoupling the px2 chain from sttA.
                    tail = len(emitted_dw) == NB
                    K1 = {}
                    for sd in sides:
                        K1[sd["src"]] = ppool.tile(
                            [128, 1, 512] if tail else [128, 2, 512], F32,
                            name=f"K1{sd['src']}",
                            tag="T" if tail else f"K{sd['src']}", bufs=1,
                        )
                    for cj in range(len(contribs)):
                        for sd in sides:
                            pw_mm_one(sd, K1[sd["src"]], 0, 2, cj)
                    P1 = {}
                    for sd in sides:
                        P1[sd["src"]] = wpool.tile(
                            [128, PXR, REG_W], DBF16,
                            name=f"P1{sd['src']}", tag=f"P1{sd['src']}", bufs=3,
                        )
                        u1 = V[(sd["other"], max(vshift, 0))][
                            :npart,
                            2 * PXR + 5 + dr : 2 * PXR + 5 + dr + PXR,
                            5 + dc : 5 + dc + REG_W,
                        ]
                        nc.vector.scalar_tensor_tensor(
                            P1[sd["src"]][:npart, :, :],
                            K1[sd["src"]][:npart, 0, 0:NPX].rearrange(
                                "p (b c) -> p b c", b=PXR
                            ),
                            sd["bias"],
                            u1,
                            ALU.add,
                            ALU.mult,
                        )
                    for sd in sides:
                        reduce_mm(sd, P1[sd["src"]][:npart, :, :], 2)

            # ---------- norm + relu -> Z
            for p in range(PXC):
                nc.scalar.activation(
                    Z[:, p * PXR : (p + 1) * PXR, :],
                    comb[p][:, :, :],
                    AF.Relu,
                    bias=small[:, SM_NORMB : SM_NORMB + 1],
                    scale=small[:, SM_NORMS : SM_NORMS + 1],
                )

            # ---------- zero the fuse-conv halo ring of Z
            nc.vector.tensor_scalar_mul(Z[:, :, 0:1], Z[:, :, 0:1], 0.0)
            nc.vector.tensor_scalar_mul(
                Z[:, :, REG_W - 1 : REG_W], Z[:, :, REG_W - 1 : REG_W], 0.0
            )
            nc.vector.tensor_scalar_mul(Z[:, 0:1, :], Z[:, 0:1, :], rmask[:, 0:1])
            nc.vector.tensor_scalar_mul(
                Z[:, REG_H - 1 : REG_H, :], Z[:, REG_H - 1 : REG_H, :], rmask[:, 1:2]
            )

            # ---------- fuse 3x3 conv (K=128), two row-halves concurrent
            osb = wpool.tile([128, TILE_H, W], F32, name="osb")
            for hh in range(2):
                fps = ppool.tile(
                    [128, 8, W], F32, name=f"fps{hh}",
                    tag=f"K{'y' if hh == 0 else 'x'}", bufs=1,
                )
                cg = 0 if hh == 0 else 64
                for ij in range(9):
                    i, j = ij // 3, ij % 3
                    rr = hh * 8 + i  # Z row of out-row (hh*8) + tap i
                    nc.tensor.matmul(
                        fps[cg : cg + 64, :, :],
                        fwb[:, ij * 64 : (ij + 1) * 64],
                        Z[:, rr : rr + 8, j : j + W],
                        start=(ij == 0),
                        stop=(ij == 8),
                        tile_position=(0, cg),
                        skip_group_check=True,
                    )
                nc.scalar.activation(
                    osb[0:64, hh * 8 : (hh + 1) * 8, :],
                    fps[cg : cg + 64, :, :],
                    AF.Relu,
                    bias=small[0:64, SM_FBNB : SM_FBNB + 1],
                    scale=small[0:64, SM_FBNS : SM_FBNS + 1],
                )
            nc.sync.dma_start(out_d[:, :, :], osb[0:64, :, :])

    nc.compile()
    return nc


# ------------------------------------------------------------------ driver
LAST_RESULTS = None


def kernel(y, x, params):
    global LAST_RESULTS
    y = np.asarray(y, np.float32)
    x = np.asarray(x, np.float32)
    packed = _pack_params(params)

    in_maps = [_core_inputs(ci, y, x, packed) for ci in range(8)]
    nc = build_nc()

    from concourse.bass_utils import run_bass_kernel_spmd

    res = run_bass_kernel_spmd(
        nc,
        in_maps,
        core_ids=list(range(8)),
        trace=bool(os.environ.get("KTRACE")),
    )
    LAST_RESULTS = res

    out = np.zeros((N_IMG, OUT_C, H, W), np.float32)
    for ci in range(8):
        n, t = ci // 4, ci % 4
        out[n, :, TILE_H * t : TILE_H * (t + 1), :] = (
            np.asarray(res.results[ci]["out"], np.float32).reshape(OUT_C, TILE_H, W)
        )
    return out
